# revision 4
# baseline (speedup 1.0000x reference)
"""Supervised contrastive loss on 8 trn2 NeuronCores (Bass/Tile).

Full inputs -> full output. Sharding: rows of the (sorted-by-label,
per-core rolled) embedding matrix are split 1024/core. Each core
computes its 1024x8192 block of the similarity matrix against the full
embedding set, reduces it to a partial loss sum; host sums the 8
partials and divides by the (host-computed) valid pair count.

Key algebra: with z_ij = exp(sim_ij) and ns_i = sum_{labels differ} z_ij,
  pair_loss_ij = logaddexp(sim_ij, log ns_i) - sim_ij = ln(1 + ns_i / z_ij)
Rows are sorted by label and rolled per-core so that every positive
(same-label) column of a core's rows lives in a fixed window [0, W),
so the ln() pass only touches W/B of the matrix. Same-label masks come
from a one-hot label matmul (K=64) on the TensorEngine.
"""

import math
import os
import sys

import numpy as np

for _p in ("/opt/trn_rl_repo", "/root/.axon_site/_ro/trn_rl_repo"):
    if os.path.isdir(_p) and _p not in sys.path:
        sys.path.append(_p)

B = 8192
D = 128
NCLS = 64
TEMP = 0.07
SCALE = 1.0 / TEMP
N_CORES = 8
R = B // N_CORES  # rows per core
P = 128  # partitions
CH = 1536  # psum G-chunk width (3 banks)


def _split_multi_waits(nc, mybir, max_waits=1):
    """Hoist excess per-instruction sync waits onto same-engine NoOps.

    This container's walrus rejects instructions carrying more than one
    sync wait ("Too many sync wait commands"); semantics are identical
    when the preceding NoOps on the same engine perform the waits.
    """
    n_new = 0
    for func in nc.m.functions:
        for block in func.blocks:
            il = block.instructions
            i = 0
            while i < len(il):
                inst = il[i]
                si = getattr(inst, "sync_info", None)
                ow = list(si.on_wait) if (si is not None and si.on_wait) else []
                if len(ow) > max_waits:
                    keep = ow[-max_waits:]
                    hoist = ow[:-max_waits]
                    nops = []
                    for w in hoist:
                        nop = mybir.InstNoOp(
                            name=f"{inst.name}-ws{len(nops)}",
                            engine=inst.engine,
                            ins=[],
                            outs=[],
                            sync_info=mybir.SyncInfo(on_wait=[w], on_update=[]),
                        )
                        nops.append(nop)
                        n_new += 1
                    inst.sync_info = mybir.SyncInfo(
                        on_wait=keep,
                        on_update=list(si.on_update) if si.on_update else [],
                    )
                    il[i:i] = nops
                    i += len(nops)
                i += 1
    return n_new


def _build_program(W: int, OFF: int):
    import concourse.bass as bass
    import concourse.tile as tile
    from concourse import mybir
    from concourse.masks import make_identity

    f32 = mybir.dt.float32
    AF = mybir.ActivationFunctionType
    OP = mybir.AluOpType

    nc = bass.Bass()
    d_emb = nc.dram_tensor("emb", [B, D], f32, kind="ExternalInput")
    d_ohw = nc.dram_tensor("ohw", [NCLS, W], f32, kind="ExternalInput")
    d_out = nc.dram_tensor("out", [1, 1], f32, kind="ExternalOutput")

    NT = B // P  # 64 row-tiles of the full matrix
    NRT = R // P  # 8 row-tiles owned by this core
    NW = W // 512  # mask tiles in window

    # chunk list for the exp sweep: non-window chunks first, window last
    nonwin = []
    c0 = W
    while c0 < B:
        cw = min(CH, B - c0)
        nonwin.append((c0, cw))
        c0 += cw
    winchunks = []
    c0 = 0
    while c0 < W:
        cw = min(CH, W - c0)
        winchunks.append((c0, cw))
        c0 += cw
    chunks = nonwin + winchunks
    NCHUNK = len(chunks)

    with tile.TileContext(nc) as tc:
        with (
            tc.tile_pool(name="big", bufs=1) as pBig,
            tc.tile_pool(name="consts", bufs=1) as pC,
            tc.tile_pool(name="norm", bufs=1) as pN,
            tc.tile_pool(name="zw", bufs=2) as pZ,
            tc.tile_pool(name="uw", bufs=2) as pU,
            tc.tile_pool(name="fw", bufs=2) as pF,
            tc.tile_pool(name="msb", bufs=2) as pM,
            tc.tile_pool(name="dump", bufs=2) as pDump,
            tc.tile_pool(name="sttv", bufs=2) as pSttV,
            tc.tile_pool(name="sttg", bufs=2) as pSttG,
            tc.tile_pool(name="sc", bufs=2) as pSc,
            tc.tile_pool(name="acc", bufs=1) as pAcc,
            tc.tile_pool(name="psG", bufs=2, space="PSUM") as psG,
            tc.tile_pool(name="psS", bufs=2, space="PSUM") as psS,
        ):
            # ---------------- load ----------------
            emb3d = pBig.tile([P, NT, D], f32, tag="emb")
            nc.sync.dma_start(
                out=emb3d, in_=d_emb[:, :].rearrange("(t p) d -> p t d", p=P)
            )
            ohw = pC.tile([NCLS, W], f32, tag="ohw")
            nc.sync.dma_start(out=ohw, in_=d_ohw[:, :])

            ident = pC.tile([P, P], f32, tag="ident")
            make_identity(nc, ident)
            ud = pC.tile([P, 1], f32, tag="ud")
            nc.vector.memset(ud, math.exp(-SCALE))
            ones = pC.tile([P, 1], f32, tag="ones")
            nc.vector.memset(ones, 1.0)
            loss_acc = pAcc.tile([P, 1], f32, tag="lacc")
            nc.vector.memset(loss_acc, 0.0)

            # ---------------- norms ----------------
            # sq shares the eT slot (dead before eT is written)
            sq3d = pBig.tile([P, NT, D], f32, tag="eT")
            nc.gpsimd.tensor_mul(sq3d, emb3d, emb3d)
            ssq = pN.tile([P, NT], f32, tag="ssq")
            nc.vector.tensor_reduce(ssq, sq3d, axis=mybir.AxisListType.X, op=OP.add)
            nc.vector.tensor_scalar_max(ssq, ssq, 1e-24)
            lnssq = pN.tile([P, NT], f32, tag="lnssq")
            nc.scalar.activation(lnssq, ssq, AF.Ln)
            inv = pN.tile([P, NT], f32, tag="inv")
            # 1/sqrt(ssq) = exp(-0.5*ln(ssq)); avoids the sqrt table set
            nc.scalar.activation(inv, lnssq, AF.Exp, scale=-0.5)

            # ------------- normalize + transpose -> eT (D x B) -------------
            eT = pBig.tile([P, B], f32, tag="eT")
            for t in range(NT):
                nc.vector.tensor_scalar_mul(
                    emb3d[:, t, :], emb3d[:, t, :], inv[:, t : t + 1]
                )
                tp = psS.tile([P, 512], f32, tag="ps_s")
                nc.tensor.transpose(tp[:, :P], emb3d[:, t, :], ident)
                nc.vector.tensor_copy(eT[:, t * P : (t + 1) * P], tp[:, :P])

            # ---------------- main loop over this core's row tiles ----------------
            for rt in range(NRT):
                row0 = OFF + rt * P
                lhsT_e = eT[:, row0 : row0 + P]
                lhsT_m = ohw[:, row0 : row0 + P]

                parts = pSc.tile([P, 24], f32, tag="parts")
                zw = pZ.tile([P, W], f32, tag="zw")

                # same-label masks for the window -> SBUF
                m_sb = pM.tile([P, W], f32, tag="m")
                for w in range(NW):
                    pm = psS.tile([P, 512], f32, tag="ps_s")
                    nc.tensor.matmul(
                        pm,
                        lhsT=lhsT_m,
                        rhs=ohw[:, w * 512 : (w + 1) * 512],
                        start=True,
                        stop=True,
                    )
                    nc.vector.tensor_copy(m_sb[:, w * 512 : (w + 1) * 512], pm)

                # exp sweep over all B columns; z kept only for the window
                for ci, (cs, cw) in enumerate(chunks):
                    g = psG.tile([P, CH], f32, tag="g")
                    for s in range(0, cw, 512):
                        sw = min(512, cw - s)
                        nc.tensor.matmul(
                            g[:, s : s + sw],
                            lhsT=lhsT_e,
                            rhs=eT[:, cs + s : cs + s + sw],
                            start=True,
                            stop=True,
                        )
                    if cs < W:  # window chunk: keep z
                        outap = zw[:, cs : cs + cw]
                    else:
                        dmp = pDump.tile([P, CH], f32, tag="dump")
                        outap = dmp[:, :cw]
                    nc.scalar.activation(
                        outap,
                        g[:, :cw],
                        AF.Exp,
                        scale=SCALE,
                        accum_out=parts[:, ci : ci + 1],
                    )

                tot = parts[:, 20:21]
                nc.vector.tensor_reduce(
                    tot, parts[:, 0:NCHUNK], axis=mybir.AxisListType.X, op=OP.add
                )

                # same-label sum over the window: sum_j z*m
                for w in range(NW):
                    sl = slice(w * 512, (w + 1) * 512)
                    dv = pSttV.tile([P, 512], f32, tag="sttv")
                    nc.vector.scalar_tensor_tensor(
                        out=dv,
                        in0=zw[:, sl],
                        scalar=1.0,
                        in1=m_sb[:, sl],
                        op0=OP.mult,
                        op1=OP.mult,
                        accum_out=parts[:, 8 + w : 9 + w],
                    )
                same = parts[:, 21:22]
                nc.vector.tensor_reduce(
                    same, parts[:, 8 : 8 + NW], axis=mybir.AxisListType.X, op=OP.add
                )
                ns = parts[:, 22:23]
                nc.vector.tensor_tensor(ns, tot, same, op=OP.subtract)

                # positive pair losses: f = ln(1 + ns/z) over the window
                uw = pU.tile([P, W], f32, tag="uw")
                nc.vector.reciprocal(uw, zw)
                fw = pF.tile([P, W], f32, tag="fw")
                nc.scalar.activation(fw, uw, AF.Ln, bias=1.0, scale=ns)
                for w in range(NW):
                    sl = slice(w * 512, (w + 1) * 512)
                    dg = pSttG.tile([P, 512], f32, tag="sttg")
                    nc.vector.scalar_tensor_tensor(
                        out=dg,
                        in0=fw[:, sl],
                        scalar=1.0,
                        in1=m_sb[:, sl],
                        op0=OP.mult,
                        op1=OP.mult,
                        accum_out=parts[:, 12 + w : 13 + w],
                    )
                posr = parts[:, 23:24]
                nc.vector.tensor_reduce(
                    posr, parts[:, 12 : 12 + NW], axis=mybir.AxisListType.X, op=OP.add
                )
                # subtract the diagonal term f_ii = ln(1 + ns*exp(-1/T))
                fd = parts[:, 16:17]
                nc.scalar.activation(fd, ud, AF.Ln, bias=1.0, scale=ns)
                rowpos = parts[:, 17:18]
                nc.vector.tensor_tensor(rowpos, posr, fd, op=OP.subtract)
                nc.vector.tensor_add(loss_acc, loss_acc, rowpos)

            # ---------------- final partition reduce + store ----------------
            pfin = psS.tile([P, 512], f32, tag="ps_s")
            nc.tensor.matmul(
                pfin[:1, :1], lhsT=loss_acc, rhs=ones, start=True, stop=True
            )
            sfin = pSc.tile([1, 1], f32, tag="sfin")
            nc.vector.tensor_copy(sfin, pfin[:1, :1])
            nc.sync.dma_start(out=d_out[:, :], in_=sfin)

    _split_multi_waits(nc, mybir)
    return nc


def _plan(labels: np.ndarray):
    """Sort-by-label order, per-core rolls, window geometry."""
    order = np.argsort(labels, kind="stable")
    counts = np.bincount(labels)
    max_cls = int(counts.max()) if counts.size else 1
    off = max(256, 128 * ((max_cls + 127) // 128))
    w = R + 2 * off
    w = 512 * ((w + 511) // 512)
    if w >= B:
        w = B
    return order, counts, off, w


def kernel(embeddings: np.ndarray, labels: np.ndarray) -> np.ndarray:
    from concourse.bass_utils import run_bass_kernel_spmd

    emb = np.ascontiguousarray(np.asarray(embeddings, dtype=np.float32))
    lab = np.asarray(labels).astype(np.int64).ravel()
    assert emb.shape == (B, D) and lab.shape == (B,)

    order, counts, off, w = _plan(lab)

    in_maps = []
    cls_ids = np.arange(NCLS, dtype=np.int64)
    for k in range(N_CORES):
        ck = np.roll(order, off - R * k)
        emb_k = np.ascontiguousarray(emb[ck])
        lab_w = lab[ck[:w]]
        ohw_k = (lab_w[None, :] == cls_ids[:, None]).astype(np.float32)
        in_maps.append({"emb": emb_k, "ohw": np.ascontiguousarray(ohw_k)})

    nc = _build_program(w, off)
    res = run_bass_kernel_spmd(nc, in_maps, core_ids=list(range(N_CORES)))
    loss_sum = float(sum(r["out"][0, 0] for r in res.results))

    n_c = counts[lab]
    valid = (n_c >= 2) & (n_c <= B - 1)
    valid_count = int((n_c - 1)[valid].sum())
    loss = loss_sum / valid_count if valid_count > 0 else 0.0
    return np.asarray([loss], dtype=np.float32)


# revision 7
# speedup vs baseline: 1.6493x; 1.6493x over previous
"""Supervised contrastive loss on 8 trn2 NeuronCores (Bass/Tile).

Full inputs -> full output. Sharding: rows of the (sorted-by-label,
per-core rolled) embedding matrix are split 1024/core. Each core
computes its 1024x8192 block of the similarity matrix against the full
embedding set in bf16 on the TensorEngine, reduces it to a partial
loss sum; host sums the 8 partials and divides by the (host-computed)
valid pair count.

Key algebra: with z_ij = exp(sim_ij) and ns_i = sum_{labels differ} z_ij,
  pair_loss_ij = logaddexp(sim_ij, log ns_i) - sim_ij
              = ln(z_ij + ns_i) - sim_ij
Rows are sorted by label and rolled per-core so that all positives
(same-label columns) of each 128-row tile live in one 512-wide window;
the ln() pass and masked reductions only touch that window. Same-label
masks are tiny and data-dependent, so they are precomputed host-side
and DMA'd in.
"""

import math
import os
import sys

import numpy as np

for _p in ("/opt/trn_rl_repo", "/root/.axon_site/_ro/trn_rl_repo"):
    if os.path.isdir(_p) and _p not in sys.path:
        sys.path.append(_p)

B = 8192
D = 128
TEMP = 0.07
SCALE = 1.0 / TEMP
N_CORES = 8
R = B // N_CORES  # rows per core
P = 128  # partitions
CH = 2048  # exp sweep chunk width (4 psum banks)
EXP_S0 = math.exp(SCALE)  # z_ii for a unit-norm row


def _split_multi_waits(nc, mybir, max_waits=1):
    """Hoist excess per-instruction sync waits onto same-engine NoOps.

    This container's walrus rejects instructions carrying more than one
    sync wait ("Too many sync wait commands"); semantics are identical
    when the preceding NoOps on the same engine perform the waits.
    """
    n_new = 0
    for func in nc.m.functions:
        for block in func.blocks:
            il = block.instructions
            i = 0
            while i < len(il):
                inst = il[i]
                si = getattr(inst, "sync_info", None)
                ow = list(si.on_wait) if (si is not None and si.on_wait) else []
                if len(ow) > max_waits:
                    keep = ow[-max_waits:]
                    hoist = ow[:-max_waits]
                    nops = []
                    for w in hoist:
                        nop = mybir.InstNoOp(
                            name=f"{inst.name}-ws{len(nops)}",
                            engine=inst.engine,
                            ins=[],
                            outs=[],
                            sync_info=mybir.SyncInfo(on_wait=[w], on_update=[]),
                        )
                        nops.append(nop)
                        n_new += 1
                    inst.sync_info = mybir.SyncInfo(
                        on_wait=keep,
                        on_update=list(si.on_update) if si.on_update else [],
                    )
                    il[i:i] = nops
                    i += len(nops)
                i += 1
    return n_new


def _build_program(WIN: int, OFF: int):
    import concourse.bass as bass
    import concourse.tile as tile
    from concourse import mybir

    f32 = mybir.dt.float32
    bf16 = mybir.dt.bfloat16
    AF = mybir.ActivationFunctionType
    OP = mybir.AluOpType

    nc = bass.Bass()
    d_emb = nc.dram_tensor("emb", [B, D], bf16, kind="ExternalInput")
    d_msk = nc.dram_tensor("msk", [P, (R // P) * WIN], bf16, kind="ExternalInput")
    d_out = nc.dram_tensor("out", [1, 1], f32, kind="ExternalOutput")

    NT = B // P  # 64 column tiles of the full matrix
    NRT = R // P  # 8 row tiles owned by this core
    NCHUNK = B // CH  # 4 sweep chunks
    half = (WIN - P) // 2  # window margin each side of the 128 rows

    with tile.TileContext(nc) as tc:
        with (
            tc.tile_pool(name="big", bufs=1) as pBig,
            tc.tile_pool(name="consts", bufs=1) as pC,
            tc.tile_pool(name="norm", bufs=1) as pN,
            tc.tile_pool(name="diag", bufs=3) as pD,
            tc.tile_pool(name="zw", bufs=2) as pZ,
            tc.tile_pool(name="fw", bufs=2) as pF,
            tc.tile_pool(name="dump", bufs=2) as pDump,
            tc.tile_pool(name="sttd", bufs=2) as pStt,
            tc.tile_pool(name="sc", bufs=2) as pSc,
            tc.tile_pool(name="acc", bufs=1) as pAcc,
            tc.tile_pool(name="ps", bufs=2, space="PSUM") as psP,
        ):
            # ---------------- load ----------------
            emb3d = pBig.tile([P, NT, D], bf16, tag="emb")
            nc.sync.dma_start(
                out=emb3d, in_=d_emb[:, :].rearrange("(t p) d -> p t d", p=P)
            )
            msk = pC.tile([P, NRT, WIN], bf16, tag="msk")
            nc.sync.dma_start(
                out=msk, in_=d_msk[:, :].rearrange("p (t w) -> p t w", w=WIN)
            )

            ud = pC.tile([P, 1], f32, tag="ud")
            nc.vector.memset(ud, 1.0)  # per-partition ones
            es0 = pC.tile([P, 1], f32, tag="es0")
            nc.vector.memset(es0, EXP_S0)
            loss_acc = pAcc.tile([P, 1], f32, tag="lacc")
            nc.vector.memset(loss_acc, 0.0)

            # ---------------- norms ----------------
            sq3d = pBig.tile([P, NT, D], f32, tag="eT")  # shares eT slot
            nc.vector.tensor_mul(sq3d, emb3d, emb3d)
            ssq = pN.tile([P, NT], f32, tag="ssq")
            nc.vector.tensor_reduce(ssq, sq3d, axis=mybir.AxisListType.X, op=OP.add)
            nc.vector.tensor_scalar_max(ssq, ssq, 1e-24)
            lnssq = pN.tile([P, NT], f32, tag="lnssq")
            nc.scalar.activation(lnssq, ssq, AF.Ln)
            inv = pN.tile([P, NT], bf16, tag="inv")
            # 1/sqrt(ssq) = exp(-0.5*ln(ssq)); avoids the sqrt table set
            nc.scalar.activation(inv, lnssq, AF.Exp, scale=-0.5)

            # ---- normalize + transpose fused: eT_tile = emb_tile.T @ diag(inv) ----
            eT = pBig.tile([P, B], bf16, tag="eT")
            PACK = CH // P  # 16 transposed tiles per psum slot
            for tg in range(NT // PACK):
                tp = psP.tile([P, CH], f32, tag="g")
                for ti in range(PACK):
                    t = tg * PACK + ti
                    dg = pD.tile([P, P], bf16, tag="dg")
                    nc.gpsimd.memset(dg, 0.0)
                    nc.gpsimd.affine_select(
                        out=dg,
                        in_=inv[:, t : t + 1].to_broadcast([P, P]),
                        compare_op=OP.is_equal,
                        fill=0.0,
                        base=0,
                        channel_multiplier=1,
                        pattern=[[-1, P]],
                    )
                    nc.tensor.matmul(
                        tp[:, ti * P : (ti + 1) * P],
                        lhsT=emb3d[:, t, :],
                        rhs=dg,
                        start=True,
                        stop=True,
                    )
                nc.vector.tensor_copy(eT[:, tg * CH : (tg + 1) * CH], tp)

            # ---------------- main loop over this core's row tiles ----------------
            for rt in range(NRT):
                row0 = OFF + rt * P
                c0 = row0 - half  # window start column
                lhsT_e = eT[:, row0 : row0 + P]
                m_rt = msk[:, rt, :]

                parts = pSc.tile([P, 16], f32, tag="parts")
                zw = pZ.tile([P, WIN], bf16, tag="zw")

                for ci in range(NCHUNK):
                    cs = ci * CH
                    g = psP.tile([P, CH], f32, tag="g")
                    for s in range(0, CH, 512):
                        nc.tensor.matmul(
                            g[:, s : s + 512],
                            lhsT=lhsT_e,
                            rhs=eT[:, cs + s : cs + s + 512],
                            start=True,
                            stop=True,
                        )
                    if ci == 0:
                        # window chunk: split exp around [c0, c0+WIN)
                        dmp = pDump.tile([P, CH], bf16, tag="dump")
                        nc.scalar.activation(
                            dmp[:, :c0],
                            g[:, :c0],
                            AF.Exp,
                            scale=SCALE,
                            accum_out=parts[:, 0:1],
                        )
                        nc.scalar.activation(
                            zw,
                            g[:, c0 : c0 + WIN],
                            AF.Exp,
                            scale=SCALE,
                            accum_out=parts[:, 1:2],
                        )
                        nc.scalar.activation(
                            dmp[:, c0 + WIN :],
                            g[:, c0 + WIN :],
                            AF.Exp,
                            scale=SCALE,
                            accum_out=parts[:, 2:3],
                        )
                        # B = sum_j m*G/T over the window, straight from PSUM
                        db = pStt.tile([P, WIN], f32, tag="sttd")
                        nc.vector.scalar_tensor_tensor(
                            out=db,
                            in0=g[:, c0 : c0 + WIN],
                            scalar=SCALE,
                            in1=m_rt,
                            op0=OP.mult,
                            op1=OP.mult,
                            accum_out=parts[:, 9:10],
                        )
                    else:
                        dmp = pDump.tile([P, CH], bf16, tag="dump")
                        nc.scalar.activation(
                            dmp,
                            g,
                            AF.Exp,
                            scale=SCALE,
                            accum_out=parts[:, ci + 2 : ci + 3],
                        )

                # same-label sum over the window: sum_j z*m
                ds = pStt.tile([P, WIN], f32, tag="sttd")
                nc.vector.scalar_tensor_tensor(
                    out=ds,
                    in0=zw,
                    scalar=1.0,
                    in1=m_rt,
                    op0=OP.mult,
                    op1=OP.mult,
                    accum_out=parts[:, 6:7],
                )
                tot = parts[:, 10:11]
                nc.vector.tensor_reduce(
                    tot,
                    parts[:, 0 : NCHUNK + 2],
                    axis=mybir.AxisListType.X,
                    op=OP.add,
                )
                ns = parts[:, 11:12]
                nc.vector.tensor_tensor(ns, tot, parts[:, 6:7], op=OP.subtract)

                # fw = ln(z + ns) over the window
                fw = pF.tile([P, WIN], bf16, tag="fw")
                nc.scalar.activation(fw, zw, AF.Ln, bias=ns, scale=1.0)
                da = pStt.tile([P, WIN], f32, tag="sttd")
                nc.vector.scalar_tensor_tensor(
                    out=da,
                    in0=fw,
                    scalar=1.0,
                    in1=m_rt,
                    op0=OP.mult,
                    op1=OP.mult,
                    accum_out=parts[:, 8:9],
                )
                # fd = ln(ns + e^{1/T}) (diagonal term of A)
                fd = parts[:, 12:13]
                nc.scalar.activation(fd, ns, AF.Ln, bias=es0, scale=1.0)
                # rowpos = (A - fd) - B + 1/T
                t1 = parts[:, 13:14]
                nc.vector.tensor_tensor(t1, parts[:, 8:9], fd, op=OP.subtract)
                t2 = parts[:, 14:15]
                nc.vector.tensor_tensor(t2, t1, parts[:, 9:10], op=OP.subtract)
                nc.vector.tensor_scalar_add(t2, t2, SCALE)
                nc.vector.tensor_add(loss_acc, loss_acc, t2)

            # ---------------- final partition reduce + store ----------------
            pfin = psP.tile([P, CH], f32, tag="g")
            nc.tensor.matmul(
                pfin[:1, :1], lhsT=loss_acc, rhs=ud, start=True, stop=True
            )
            sfin = pSc.tile([1, 1], f32, tag="sfin")
            nc.vector.tensor_copy(sfin, pfin[:1, :1])
            nc.sync.dma_start(out=d_out[:, :], in_=sfin)

    _split_multi_waits(nc, mybir)
    return nc


def _plan(labels: np.ndarray):
    """Sort-by-label order, window geometry."""
    order = np.argsort(labels, kind="stable")
    counts = np.bincount(labels)
    max_cls = int(counts.max()) if counts.size else 1
    # per-row-tile window: 128 rows + margin >= max_cls-1 each side
    win = 512
    while win < B and (win - P) // 2 < max_cls - 1:
        win += 512
    win = min(win, 2048)  # window must fit inside sweep chunk 0
    off = max(256, (win - P) // 2 + 64)
    assert (win - P) // 2 >= max_cls - 1 or win == 2048, "class too large"
    return order, counts, off, win


def _host_inputs(emb, lab, order, off, win):
    import ml_dtypes

    half = (win - P) // 2
    emb_bf = emb.astype(ml_dtypes.bfloat16)
    in_maps = []
    for k in range(N_CORES):
        ck = np.roll(order, off - R * k)
        lab_r = lab[ck]
        # per-row-tile same-label masks over each tile's window
        m = np.zeros((P, R // P, win), dtype=np.float32)
        for rt in range(R // P):
            row0 = off + rt * P
            c0 = row0 - half
            rl = lab_r[row0 : row0 + P]
            cl = lab_r[c0 : c0 + win]
            m[:, rt, :] = rl[:, None] == cl[None, :]
        in_maps.append(
            {
                "emb": np.ascontiguousarray(emb_bf[ck]),
                "msk": np.ascontiguousarray(
                    m.reshape(P, -1).astype(ml_dtypes.bfloat16)
                ),
            }
        )
    return in_maps


def kernel(embeddings: np.ndarray, labels: np.ndarray) -> np.ndarray:
    from concourse.bass_utils import run_bass_kernel_spmd

    emb = np.ascontiguousarray(np.asarray(embeddings, dtype=np.float32))
    lab = np.asarray(labels).astype(np.int64).ravel()
    assert emb.shape == (B, D) and lab.shape == (B,)

    order, counts, off, win = _plan(lab)
    in_maps = _host_inputs(emb, lab, order, off, win)

    nc = _build_program(win, off)
    res = run_bass_kernel_spmd(nc, in_maps, core_ids=list(range(N_CORES)))
    loss_sum = float(sum(r["out"][0, 0] for r in res.results))

    n_c = counts[lab]
    valid = (n_c >= 2) & (n_c <= B - 1)
    valid_count = int((n_c - 1)[valid].sum())
    loss = loss_sum / valid_count if valid_count > 0 else 0.0
    return np.asarray([loss], dtype=np.float32)
